# revision 7
# baseline (speedup 1.0000x reference)
"""Trainium2 Bass kernel for nn_AggregateAttention.

Reference computation (B=64, A=6, R=29, D=512, N=1000):
    x_wx[b,a,r,e] = sum_d x[b,r,d] * wx[a,r,d,e] + wx_bias[a,r,e]
    y_wy[r,n,e]   = sum_d y[r,n,d] * wy[r,d,e] + wy_bias[r,e]
    S[b,a,r,n]    = sum_e x_wx[b,a,r,e] * y_wy[r,n,e] / sqrt(D)
    P             = softmax_n(S)
    out[b,r,a,d]  = sum_n P[b,a,r,n] * y[r,n,d]

Sharding: regions r are fully independent -> distribute 29 regions over
8 cores as 4 region-slots each (32 slots, 3 padded duplicates).  Each
core computes its slots start-to-finish; the host concatenates region
outputs.  All matmuls contract over d/e/n with the contraction dim on
SBUF partitions; scores are computed transposed (S^T [n, ba]) so the
softmax denominator is a ones-vector matmul and no bulk transposes are
needed.  Softmax normalization is folded into the output-evacuation
copy (tensor_scalar mult by 1/rowsum, transposed to partition-major via
a single tiny PE transpose per slot).

Dtypes: logit-path inputs (wx, x, wy, pool^T) are cast to bf16 on the
host (feeds softmax logits only); everything else runs as float32r
(TF32-like, ~11 mantissa bits, full PE speed at free-dim >= 256).  The
pool tensor used for the output weighted sum stays f32r.  Set
LOGITS_BF16 = False for an all-f32r variant (~3.5e-4 max rel err vs
~3e-3 with bf16 logits).
"""

from contextlib import ExitStack

import numpy as np
import ml_dtypes

import concourse.bass as bass
import concourse.tile as tile
from concourse import bacc, masks, mybir
from concourse.bass_utils import run_bass_kernel_spmd

B, A, R, D, N = 64, 6, 29, 512, 1000
N_CORES = 8
SLOTS = 4  # region slots per core (8*4 = 32 >= 29)
DC = D // 128  # 4 d/e chunks
NK = 8  # n chunks (7 full + 1 of 104)
NLAST = N - 7 * 128  # 104
BA = A * B  # 384
SCALE = float(1.0 / np.sqrt(D))

LOGITS_BF16 = True

dt = mybir.dt
_LDT = dt.bfloat16 if LOGITS_BF16 else dt.float32r
_LNP = ml_dtypes.bfloat16 if LOGITS_BF16 else np.float32

_CACHE = {}


def _np(x):
    return np.ascontiguousarray(np.asarray(x))


def _build():
    nc = bacc.Bacc(
        "TRN2", target_bir_lowering=False, debug=False, num_devices=N_CORES
    )
    wx_d = nc.dram_tensor("wx", [SLOTS, D, A, D], _LDT, kind="ExternalInput").ap()
    xt_d = nc.dram_tensor("xt", [SLOTS, D, B], _LDT, kind="ExternalInput").ap()
    pool_d = nc.dram_tensor(
        "pool", [SLOTS, N, D], dt.float32r, kind="ExternalInput"
    ).ap()
    poolt_d = nc.dram_tensor("poolt", [SLOTS, D, N], _LDT, kind="ExternalInput").ap()
    wy_d = nc.dram_tensor("wy", [SLOTS, D, D], _LDT, kind="ExternalInput").ap()
    bias_d = nc.dram_tensor(
        "bias", [SLOTS, 128, DC * A + DC], dt.float32, kind="ExternalInput"
    ).ap()
    ones_d = nc.dram_tensor("ones", [128, 1], dt.float32r, kind="ExternalInput").ap()
    out_d = nc.dram_tensor(
        "out", [B, SLOTS, A, D], dt.float32, kind="ExternalOutput"
    ).ap()

    with tile.TileContext(nc) as tc:
        with ExitStack() as ctx:
            p_const = ctx.enter_context(tc.tile_pool(name="p_const", bufs=1))
            p_wx = ctx.enter_context(tc.tile_pool(name="p_wx", bufs=2))
            p_xt = ctx.enter_context(tc.tile_pool(name="p_xt", bufs=2))
            p_pool = ctx.enter_context(tc.tile_pool(name="p_pool", bufs=2))
            p_poolt = ctx.enter_context(tc.tile_pool(name="p_poolt", bufs=2))
            p_wy = ctx.enter_context(tc.tile_pool(name="p_wy", bufs=2))
            p_bias = ctx.enter_context(tc.tile_pool(name="p_bias", bufs=2))
            p_xwxt = ctx.enter_context(tc.tile_pool(name="p_xwxt", bufs=2))
            p_ywyt = ctx.enter_context(tc.tile_pool(name="p_ywyt", bufs=1))
            p_expst = ctx.enter_context(tc.tile_pool(name="p_expst", bufs=1))
            p_soft = ctx.enter_context(tc.tile_pool(name="p_soft", bufs=2))
            p_out = ctx.enter_context(tc.tile_pool(name="p_out", bufs=2))
            ps_x = ctx.enter_context(tc.tile_pool(name="ps_x", bufs=2, space="PSUM"))
            ps_y = ctx.enter_context(tc.tile_pool(name="ps_y", bufs=2, space="PSUM"))
            ps_s = ctx.enter_context(tc.tile_pool(name="ps_s", bufs=2, space="PSUM"))
            ps_o = ctx.enter_context(tc.tile_pool(name="ps_o", bufs=2, space="PSUM"))

            ones_sb = p_const.tile([128, 1], dt.float32r)
            nc.sync.dma_start(ones_sb[:], ones_d[:])
            one_f32 = p_const.tile([1, 1], dt.float32)
            nc.vector.memset(one_f32[:], 1.0)

            for s in range(SLOTS):
                # ---- loads: B-stage inputs first (rings are FIFO), wx last ----
                xt_sb = p_xt.tile([128, DC, B], _LDT, tag="xt")
                nc.scalar.dma_start(
                    xt_sb[:], xt_d[s].rearrange("(c p) b -> p c b", p=128)
                )
                wy_sb = p_wy.tile([128, DC, D], _LDT, tag="wy")
                nc.scalar.dma_start(
                    wy_sb[:], wy_d[s].rearrange("(c p) e -> p c e", p=128)
                )
                poolt_sb = p_poolt.tile([128, DC, N], _LDT, tag="poolt")
                nc.scalar.dma_start(
                    poolt_sb[:], poolt_d[s].rearrange("(c p) n -> p c n", p=128)
                )
                bias_sb = p_bias.tile([128, DC * A + DC], dt.float32, tag="bias")
                nc.sync.dma_start(bias_sb[:], bias_d[s])
                pool_sb = p_pool.tile([128, NK, D], dt.float32r, tag="pool")
                nc.sync.dma_start(
                    pool_sb[:, 0:7, :],
                    pool_d[s, 0 : 7 * 128].rearrange("(c p) d -> p c d", p=128),
                )
                nc.sync.dma_start(pool_sb[0:NLAST, 7, :], pool_d[s, 7 * 128 : N])
                wx_sb = p_wx.tile([128, DC, A, D], _LDT, tag="wx")
                half = DC // 2
                nc.sync.dma_start(
                    wx_sb[:, 0:half],
                    wx_d[s, 0 : half * 128].rearrange("(c p) a e -> p c a e", p=128),
                )
                nc.scalar.dma_start(
                    wx_sb[:, half:DC],
                    wx_d[s, half * 128 : D].rearrange("(c p) a e -> p c a e", p=128),
                )

                def wxb(ec, a):
                    return bias_sb[:, ec * A + a : ec * A + a + 1]

                def wyb(ec):
                    return bias_sb[:, DC * A + ec : DC * A + ec + 1]

                # ---- B: y_wyT[e, n] = wy.T @ pool.T + bias ----
                ywyt_sb = p_ywyt.tile([128, DC, N], dt.float32r, tag="ywyt")
                for ec in range(DC):
                    for nh in range(2):
                        nsl = slice(nh * 500, (nh + 1) * 500)
                        psy = ps_y.tile([128, 500], dt.float32, tag="psy")
                        for c in range(DC):
                            nc.tensor.matmul(
                                psy[:],
                                wy_sb[:, c, ec * 128 : (ec + 1) * 128],
                                poolt_sb[:, c, nsl],
                                start=(c == 0),
                                stop=(c == DC - 1),
                            )
                        nc.vector.tensor_scalar_add(
                            ywyt_sb[:, ec, nsl], psy[:], wyb(ec)
                        )

                # ---- A: x_wxT[e, (a b)] = wx[a].T @ x.T + bias ----
                xwxt_sb = p_xwxt.tile([128, DC, A, B], dt.float32r, tag="xwxt")
                for ec in range(DC):
                    for a in range(A):
                        psx = ps_x.tile([128, B], dt.float32, tag="psx")
                        for c in range(DC):
                            nc.tensor.matmul(
                                psx[:],
                                wx_sb[:, c, a, ec * 128 : (ec + 1) * 128],
                                xt_sb[:, c, :],
                                start=(c == 0),
                                stop=(c == DC - 1),
                            )
                        nc.vector.tensor_scalar_add(
                            xwxt_sb[:, ec, a, :], psx[:], wxb(ec, a)
                        )

                # ---- C: S^T[n, ba] + exp (unnormalized) + rowsum accum ----
                expst_sb = p_expst.tile([128, NK, BA], dt.float32r, tag="expst")
                for nk in range(NK):
                    pp = 128 if nk < 7 else NLAST
                    pss = ps_s.tile([128, BA], dt.float32, tag="pss")
                    for ec in range(DC):
                        nc.tensor.matmul(
                            pss[0:pp, :],
                            ywyt_sb[:, ec, nk * 128 : nk * 128 + pp],
                            xwxt_sb[:, ec, :, :],
                            start=(ec == 0),
                            stop=(ec == DC - 1),
                        )
                    nc.scalar.activation(
                        expst_sb[0:pp, nk, :],
                        pss[0:pp, :],
                        mybir.ActivationFunctionType.Exp,
                        scale=SCALE,
                    )

                # ---- D: rowsums over n via ones matmul ----
                psr = ps_x.tile([1, BA], dt.float32, tag="psx")
                for nk in range(NK):
                    pp = 128 if nk < 7 else NLAST
                    nc.tensor.matmul(
                        psr[:],
                        ones_sb[0:pp, :],
                        expst_sb[0:pp, nk, :],
                        start=(nk == 0),
                        stop=(nk == NK - 1),
                    )

                # ---- D: reciprocal -> partition-major ----
                rinv = p_soft.tile([1, BA], dt.float32, tag="rinv")
                nc.vector.reciprocal(rinv[:], psr[:])
                rsc = p_soft.tile([128, 3], dt.float32, tag="rsc")
                for bc in range(3):
                    pst = ps_x.tile([128, 1], dt.float32, tag="psx")
                    nc.tensor.transpose(
                        pst[:], rinv[0:1, bc * 128 : (bc + 1) * 128], one_f32[:]
                    )
                    nc.vector.tensor_copy(rsc[:, bc : bc + 1], pst[:])

                # ---- E: out[(a b), d] = (expS^T.T @ pool) * rinv ----
                outsb = p_out.tile([128, 3, D], dt.float32, tag="outsb")
                for bc in range(3):
                    pso = ps_o.tile([128, D], dt.float32, tag="pso")
                    for nk in range(NK):
                        pp = 128 if nk < 7 else NLAST
                        nc.tensor.matmul(
                            pso[:],
                            expst_sb[0:pp, nk, bc * 128 : (bc + 1) * 128],
                            pool_sb[0:pp, nk, :],
                            start=(nk == 0),
                            stop=(nk == NK - 1),
                        )
                    nc.vector.tensor_scalar_mul(
                        outsb[:, bc, :], pso[:], rsc[:, bc : bc + 1]
                    )
                    nc.scalar.dma_start(
                        out_d[:, s, 2 * bc : 2 * bc + 2, :].rearrange(
                            "b a d -> a b d"
                        ),
                        outsb[:, bc, :],
                    )

    nc.compile()
    return nc


def _prep_inputs(
    top_region_features, normality_pool_image_features, wx, wx_bias, wy, wy_bias
):
    x = _np(top_region_features).astype(np.float32)
    pool = _np(normality_pool_image_features).astype(np.float32)
    wx = _np(wx).astype(np.float32)
    wxb = _np(wx_bias).astype(np.float32)
    wy = _np(wy).astype(np.float32)
    wyb = _np(wy_bias).astype(np.float32)

    rmap = list(range(R)) + [0, 1, 2]
    ones = np.ones((128, 1), np.float32)
    in_maps = []
    for i in range(N_CORES):
        regs = rmap[SLOTS * i : SLOTS * (i + 1)]
        # bias pack: [slot, partition(=e%128), DC*A (wx) + DC (wy)]
        wxb_s = (
            wxb[:, regs, 0, :]  # (A, S, D)
            .transpose(1, 2, 0)  # (S, D, A)
            .reshape(SLOTS, DC, 128, A)
            .transpose(0, 2, 1, 3)  # (S, 128, DC, A)
            .reshape(SLOTS, 128, DC * A)
        )
        wyb_s = (
            wyb[regs, 0, :].reshape(SLOTS, DC, 128).transpose(0, 2, 1)
        )  # (S, 128, DC)
        bias = np.concatenate([wxb_s, wyb_s], axis=2)  # (S, 128, DC*A+DC)
        in_maps.append(
            {
                "wx": _np(wx[:, regs].transpose(1, 2, 0, 3)).astype(_LNP),
                "xt": _np(x[:, regs].transpose(1, 2, 0)).astype(_LNP),
                "pool": _np(pool[regs]),
                "poolt": _np(pool[regs].transpose(0, 2, 1)).astype(_LNP),
                "wy": _np(wy[regs]).astype(_LNP),
                "bias": _np(bias),
                "ones": ones,
            }
        )
    return in_maps


def _gather(results):
    full = np.empty((B, R, A, D), np.float32)
    for j in range(R):
        full[:, j] = results[j // SLOTS]["out"][:, j % SLOTS]
    return full


def _get_nc():
    if "nc" not in _CACHE:
        _CACHE["nc"] = _build()
    return _CACHE["nc"]


def run(in_maps, **kw):
    nc = _get_nc()
    return run_bass_kernel_spmd(nc, in_maps, list(range(N_CORES)), **kw)


def kernel(**inputs):
    in_maps = _prep_inputs(**inputs)
    res = run(in_maps)
    return _gather(res.results)


if __name__ == "__main__":
    rng = np.random.default_rng(0)
    s = 1.0 / np.sqrt(D)
    inputs = {
        "top_region_features": rng.standard_normal((B, R, D), np.float32),
        "normality_pool_image_features": rng.standard_normal((R, N, D), np.float32),
        "wx": (rng.standard_normal((A, R, D, D)) * s).astype(np.float32),
        "wx_bias": (rng.standard_normal((A, R, 1, D)) * s).astype(np.float32),
        "wy": (rng.standard_normal((R, D, D)) * s).astype(np.float32),
        "wy_bias": (rng.standard_normal((R, 1, D)) * s).astype(np.float32),
    }
    out = kernel(**inputs)
    print("out", out.shape, out.dtype, float(np.abs(out).mean()))


# revision 8
# speedup vs baseline: 1.0267x; 1.0267x over previous
"""Trainium2 Bass kernel for nn_AggregateAttention.

Reference computation (B=64, A=6, R=29, D=512, N=1000):
    x_wx[b,a,r,e] = sum_d x[b,r,d] * wx[a,r,d,e] + wx_bias[a,r,e]
    y_wy[r,n,e]   = sum_d y[r,n,d] * wy[r,d,e] + wy_bias[r,e]
    S[b,a,r,n]    = sum_e x_wx[b,a,r,e] * y_wy[r,n,e] / sqrt(D)
    P             = softmax_n(S)
    out[b,r,a,d]  = sum_n P[b,a,r,n] * y[r,n,d]

Sharding: regions r are fully independent -> distribute 29 regions over
8 cores as 4 region-slots each (32 slots, 3 padded duplicates).  Each
core computes its slots start-to-finish; the host concatenates region
outputs.  All matmuls contract over d/e/n with the contraction dim on
SBUF partitions; scores are computed transposed (S^T [n, ba]) so the
softmax denominator is a ones-vector matmul and no bulk transposes are
needed.  Softmax normalization is folded into the output-evacuation
copy (tensor_scalar mult by 1/rowsum, transposed to partition-major via
a single tiny PE transpose per slot).

Dtypes: logit-path inputs (wx, x, wy, pool^T) are cast to bf16 on the
host (feeds softmax logits only); everything else runs as float32r
(TF32-like, ~11 mantissa bits, full PE speed at free-dim >= 256).  The
pool tensor used for the output weighted sum stays f32r.  Set
LOGITS_BF16 = False for an all-f32r variant (~3.5e-4 max rel err vs
~3e-3 with bf16 logits).
"""

from contextlib import ExitStack

import numpy as np
import ml_dtypes

import concourse.bass as bass
import concourse.tile as tile
from concourse import bacc, masks, mybir
from concourse.bass_utils import run_bass_kernel_spmd

B, A, R, D, N = 64, 6, 29, 512, 1000
N_CORES = 8
SLOTS = 4  # region slots per core (8*4 = 32 >= 29)
DC = D // 128  # 4 d/e chunks
NK = 8  # n chunks (7 full + 1 of 104)
NLAST = N - 7 * 128  # 104
BA = A * B  # 384
SCALE = float(1.0 / np.sqrt(D))

LOGITS_BF16 = True

dt = mybir.dt
_LDT = dt.bfloat16 if LOGITS_BF16 else dt.float32r
_LNP = ml_dtypes.bfloat16 if LOGITS_BF16 else np.float32

_CACHE = {}


def _np(x):
    return np.ascontiguousarray(np.asarray(x))


def _build():
    nc = bacc.Bacc(
        "TRN2", target_bir_lowering=False, debug=False, num_devices=N_CORES
    )
    wx_d = nc.dram_tensor("wx", [SLOTS, D, A, D], _LDT, kind="ExternalInput").ap()
    xt_d = nc.dram_tensor("xt", [SLOTS, D, B], _LDT, kind="ExternalInput").ap()
    pool_d = nc.dram_tensor(
        "pool", [SLOTS, N, D], dt.float32r, kind="ExternalInput"
    ).ap()
    poolt_d = nc.dram_tensor("poolt", [SLOTS, D, N], _LDT, kind="ExternalInput").ap()
    wy_d = nc.dram_tensor("wy", [SLOTS, D, D], _LDT, kind="ExternalInput").ap()
    bias_d = nc.dram_tensor(
        "bias", [SLOTS, 128, DC * A + DC], dt.float32, kind="ExternalInput"
    ).ap()
    ones_d = nc.dram_tensor("ones", [128, 1], dt.float32r, kind="ExternalInput").ap()
    out_d = nc.dram_tensor(
        "out", [B, SLOTS, A, D], dt.float32, kind="ExternalOutput"
    ).ap()

    with tile.TileContext(nc) as tc:
        with ExitStack() as ctx:
            p_const = ctx.enter_context(tc.tile_pool(name="p_const", bufs=1))
            p_wx = ctx.enter_context(tc.tile_pool(name="p_wx", bufs=2))
            p_xt = ctx.enter_context(tc.tile_pool(name="p_xt", bufs=2))
            p_pool = ctx.enter_context(tc.tile_pool(name="p_pool", bufs=2))
            p_poolt = ctx.enter_context(tc.tile_pool(name="p_poolt", bufs=2))
            p_wy = ctx.enter_context(tc.tile_pool(name="p_wy", bufs=2))
            p_bias = ctx.enter_context(tc.tile_pool(name="p_bias", bufs=2))
            p_xwxt = ctx.enter_context(tc.tile_pool(name="p_xwxt", bufs=2))
            p_ywyt = ctx.enter_context(tc.tile_pool(name="p_ywyt", bufs=1))
            p_expst = ctx.enter_context(tc.tile_pool(name="p_expst", bufs=1))
            p_soft = ctx.enter_context(tc.tile_pool(name="p_soft", bufs=2))
            p_out = ctx.enter_context(tc.tile_pool(name="p_out", bufs=2))
            ps_x = ctx.enter_context(tc.tile_pool(name="ps_x", bufs=2, space="PSUM"))
            ps_y = ctx.enter_context(tc.tile_pool(name="ps_y", bufs=2, space="PSUM"))
            ps_s = ctx.enter_context(tc.tile_pool(name="ps_s", bufs=2, space="PSUM"))
            ps_r = ctx.enter_context(tc.tile_pool(name="ps_r", bufs=1, space="PSUM"))
            ps_o = ctx.enter_context(tc.tile_pool(name="ps_o", bufs=1, space="PSUM"))

            ones_sb = p_const.tile([128, 1], dt.float32r)
            nc.sync.dma_start(ones_sb[:], ones_d[:])
            one_f32 = p_const.tile([1, 1], dt.float32)
            nc.vector.memset(one_f32[:], 1.0)

            for s in range(SLOTS):
                # ---- loads: B-stage inputs first (rings are FIFO), wx last ----
                xt_sb = p_xt.tile([128, DC, B], _LDT, tag="xt")
                nc.scalar.dma_start(
                    xt_sb[:], xt_d[s].rearrange("(c p) b -> p c b", p=128)
                )
                wy_sb = p_wy.tile([128, DC, D], _LDT, tag="wy")
                nc.scalar.dma_start(
                    wy_sb[:], wy_d[s].rearrange("(c p) e -> p c e", p=128)
                )
                poolt_sb = p_poolt.tile([128, DC, N], _LDT, tag="poolt")
                nc.scalar.dma_start(
                    poolt_sb[:], poolt_d[s].rearrange("(c p) n -> p c n", p=128)
                )
                bias_sb = p_bias.tile([128, DC * A + DC], dt.float32, tag="bias")
                nc.sync.dma_start(bias_sb[:], bias_d[s])
                pool_sb = p_pool.tile([128, NK, D], dt.float32r, tag="pool")
                nc.sync.dma_start(
                    pool_sb[:, 0:7, :],
                    pool_d[s, 0 : 7 * 128].rearrange("(c p) d -> p c d", p=128),
                )
                nc.sync.dma_start(pool_sb[0:NLAST, 7, :], pool_d[s, 7 * 128 : N])
                wx_sb = p_wx.tile([128, DC, A, D], _LDT, tag="wx")
                half = DC // 2
                nc.sync.dma_start(
                    wx_sb[:, 0:half],
                    wx_d[s, 0 : half * 128].rearrange("(c p) a e -> p c a e", p=128),
                )
                nc.scalar.dma_start(
                    wx_sb[:, half:DC],
                    wx_d[s, half * 128 : D].rearrange("(c p) a e -> p c a e", p=128),
                )

                def wxb(ec, a):
                    return bias_sb[:, ec * A + a : ec * A + a + 1]

                def wyb(ec):
                    return bias_sb[:, DC * A + ec : DC * A + ec + 1]

                # ---- A: x_wxT[e, (a b)] = wx[a].T @ x.T + bias ----
                xwxt_sb = p_xwxt.tile([128, DC, A, B], dt.float32r, tag="xwxt")
                for ec in range(DC):
                    for a in range(A):
                        psx = ps_x.tile([128, B], dt.float32, tag="psx")
                        for c in range(DC):
                            nc.tensor.matmul(
                                psx[:],
                                wx_sb[:, c, a, ec * 128 : (ec + 1) * 128],
                                xt_sb[:, c, :],
                                start=(c == 0),
                                stop=(c == DC - 1),
                            )
                        nc.vector.tensor_scalar_add(
                            xwxt_sb[:, ec, a, :], psx[:], wxb(ec, a)
                        )

                # ---- B: y_wyT[e, n] = wy.T @ pool.T + bias ----
                ywyt_sb = p_ywyt.tile([128, DC, N], dt.float32r, tag="ywyt")
                for ec in range(DC):
                    for nh in range(2):
                        nsl = slice(nh * 500, (nh + 1) * 500)
                        psy = ps_y.tile([128, 500], dt.float32, tag="psy")
                        for c in range(DC):
                            nc.tensor.matmul(
                                psy[:],
                                wy_sb[:, c, ec * 128 : (ec + 1) * 128],
                                poolt_sb[:, c, nsl],
                                start=(c == 0),
                                stop=(c == DC - 1),
                            )
                        nc.vector.tensor_scalar_add(
                            ywyt_sb[:, ec, nsl], psy[:], wyb(ec)
                        )

                # ---- C: S^T[n, ba] + exp (unnormalized) + rowsum accum ----
                expst_sb = p_expst.tile([128, NK, BA], dt.float32r, tag="expst")
                for nk in range(NK):
                    pp = 128 if nk < 7 else NLAST
                    pss = ps_s.tile([128, BA], dt.float32, tag="pss")
                    for ec in range(DC):
                        nc.tensor.matmul(
                            pss[0:pp, :],
                            ywyt_sb[:, ec, nk * 128 : nk * 128 + pp],
                            xwxt_sb[:, ec, :, :],
                            start=(ec == 0),
                            stop=(ec == DC - 1),
                        )
                    nc.scalar.activation(
                        expst_sb[0:pp, nk, :],
                        pss[0:pp, :],
                        mybir.ActivationFunctionType.Exp,
                        scale=SCALE,
                    )

                # ---- D: rowsums over n via ones matmul ----
                psr = ps_r.tile([1, BA], dt.float32, tag="psr")
                for nk in range(NK):
                    pp = 128 if nk < 7 else NLAST
                    nc.tensor.matmul(
                        psr[:],
                        ones_sb[0:pp, :],
                        expst_sb[0:pp, nk, :],
                        start=(nk == 0),
                        stop=(nk == NK - 1),
                    )

                # ---- D: reciprocal -> partition-major ----
                rinv = p_soft.tile([1, BA], dt.float32, tag="rinv")
                nc.vector.reciprocal(rinv[:], psr[:])
                rsc = p_soft.tile([128, 3], dt.float32, tag="rsc")
                for bc in range(3):
                    pst = ps_r.tile([128, 1], dt.float32, tag="psr")
                    nc.tensor.transpose(
                        pst[:], rinv[0:1, bc * 128 : (bc + 1) * 128], one_f32[:]
                    )
                    nc.vector.tensor_copy(rsc[:, bc : bc + 1], pst[:])

                # ---- E: out[(a b), d] = (expS^T.T @ pool) * rinv ----
                outsb = p_out.tile([128, 3, D], dt.float32, tag="outsb")
                for bc in range(3):
                    pso = ps_o.tile([128, D], dt.float32, tag="pso")
                    for nk in range(NK):
                        pp = 128 if nk < 7 else NLAST
                        nc.tensor.matmul(
                            pso[:],
                            expst_sb[0:pp, nk, bc * 128 : (bc + 1) * 128],
                            pool_sb[0:pp, nk, :],
                            start=(nk == 0),
                            stop=(nk == NK - 1),
                        )
                    nc.vector.tensor_scalar_mul(
                        outsb[:, bc, :], pso[:], rsc[:, bc : bc + 1]
                    )
                    nc.scalar.dma_start(
                        out_d[:, s, 2 * bc : 2 * bc + 2, :].rearrange(
                            "b a d -> a b d"
                        ),
                        outsb[:, bc, :],
                    )

    nc.compile()
    return nc


def _prep_inputs(
    top_region_features, normality_pool_image_features, wx, wx_bias, wy, wy_bias
):
    x = _np(top_region_features).astype(np.float32)
    pool = _np(normality_pool_image_features).astype(np.float32)
    wx = _np(wx).astype(np.float32)
    wxb = _np(wx_bias).astype(np.float32)
    wy = _np(wy).astype(np.float32)
    wyb = _np(wy_bias).astype(np.float32)

    rmap = list(range(R)) + [0, 1, 2]
    ones = np.ones((128, 1), np.float32)
    in_maps = []
    for i in range(N_CORES):
        regs = rmap[SLOTS * i : SLOTS * (i + 1)]
        # bias pack: [slot, partition(=e%128), DC*A (wx) + DC (wy)]
        wxb_s = (
            wxb[:, regs, 0, :]  # (A, S, D)
            .transpose(1, 2, 0)  # (S, D, A)
            .reshape(SLOTS, DC, 128, A)
            .transpose(0, 2, 1, 3)  # (S, 128, DC, A)
            .reshape(SLOTS, 128, DC * A)
        )
        wyb_s = (
            wyb[regs, 0, :].reshape(SLOTS, DC, 128).transpose(0, 2, 1)
        )  # (S, 128, DC)
        bias = np.concatenate([wxb_s, wyb_s], axis=2)  # (S, 128, DC*A+DC)
        in_maps.append(
            {
                "wx": _np(wx[:, regs].transpose(1, 2, 0, 3)).astype(_LNP),
                "xt": _np(x[:, regs].transpose(1, 2, 0)).astype(_LNP),
                "pool": _np(pool[regs]),
                "poolt": _np(pool[regs].transpose(0, 2, 1)).astype(_LNP),
                "wy": _np(wy[regs]).astype(_LNP),
                "bias": _np(bias),
                "ones": ones,
            }
        )
    return in_maps


def _gather(results):
    full = np.empty((B, R, A, D), np.float32)
    for j in range(R):
        full[:, j] = results[j // SLOTS]["out"][:, j % SLOTS]
    return full


def _get_nc():
    if "nc" not in _CACHE:
        _CACHE["nc"] = _build()
    return _CACHE["nc"]


def run(in_maps, **kw):
    nc = _get_nc()
    return run_bass_kernel_spmd(nc, in_maps, list(range(N_CORES)), **kw)


def kernel(**inputs):
    in_maps = _prep_inputs(**inputs)
    res = run(in_maps)
    return _gather(res.results)


if __name__ == "__main__":
    rng = np.random.default_rng(0)
    s = 1.0 / np.sqrt(D)
    inputs = {
        "top_region_features": rng.standard_normal((B, R, D), np.float32),
        "normality_pool_image_features": rng.standard_normal((R, N, D), np.float32),
        "wx": (rng.standard_normal((A, R, D, D)) * s).astype(np.float32),
        "wx_bias": (rng.standard_normal((A, R, 1, D)) * s).astype(np.float32),
        "wy": (rng.standard_normal((R, D, D)) * s).astype(np.float32),
        "wy_bias": (rng.standard_normal((R, 1, D)) * s).astype(np.float32),
    }
    out = kernel(**inputs)
    print("out", out.shape, out.dtype, float(np.abs(out).mean()))


# revision 9
# speedup vs baseline: 1.1969x; 1.1657x over previous
"""Trainium2 Bass kernel for nn_AggregateAttention.

Reference computation (B=64, A=6, R=29, D=512, N=1000):
    x_wx[b,a,r,e] = sum_d x[b,r,d] * wx[a,r,d,e] + wx_bias[a,r,e]
    y_wy[r,n,e]   = sum_d y[r,n,d] * wy[r,d,e] + wy_bias[r,e]
    S[b,a,r,n]    = sum_e x_wx[b,a,r,e] * y_wy[r,n,e] / sqrt(D)
    P             = softmax_n(S)
    out[b,r,a,d]  = sum_n P[b,a,r,n] * y[r,n,d]

Sharding: regions r are fully independent -> distribute 29 regions over
8 cores as 4 region-slots each (32 slots, 3 padded duplicates).  Each
core computes its slots start-to-finish; the host concatenates region
outputs.  All matmuls contract over d/e/n with the contraction dim on
SBUF partitions; scores are computed transposed (S^T [n, ba]) so the
softmax denominator is a ones-vector matmul and no bulk transposes are
needed.  Softmax normalization is folded into the output-evacuation
copy (tensor_scalar mult by 1/rowsum, transposed to partition-major via
a single tiny PE transpose per slot).

Dtypes: logit-path inputs (wx, x, wy, pool^T) are cast to bf16 on the
host (feeds softmax logits only); everything else runs as float32r
(TF32-like, ~11 mantissa bits, full PE speed at free-dim >= 256).  The
pool tensor used for the output weighted sum stays f32r.  Set
LOGITS_BF16 = False for an all-f32r variant (~3.5e-4 max rel err vs
~3e-3 with bf16 logits).
"""

from contextlib import ExitStack

import numpy as np
import ml_dtypes

import concourse.bass as bass
import concourse.tile as tile
from concourse import bacc, masks, mybir
from concourse.bass_utils import run_bass_kernel_spmd

B, A, R, D, N = 64, 6, 29, 512, 1000
N_CORES = 8
SLOTS = 4  # region slots per core (8*4 = 32 >= 29)
DC = D // 128  # 4 d/e chunks
NK = 8  # n chunks (7 full + 1 of 104)
NLAST = N - 7 * 128  # 104
BA = A * B  # 384
SCALE = float(1.0 / np.sqrt(D))

LOGITS_BF16 = True

dt = mybir.dt
_LDT = dt.bfloat16 if LOGITS_BF16 else dt.float32r
_LNP = ml_dtypes.bfloat16 if LOGITS_BF16 else np.float32

_CACHE = {}


def _np(x):
    return np.ascontiguousarray(np.asarray(x))


def _build():
    nc = bacc.Bacc(
        "TRN2", target_bir_lowering=False, debug=False, num_devices=N_CORES
    )
    wx_d = nc.dram_tensor("wx", [SLOTS, D, A, D], _LDT, kind="ExternalInput").ap()
    xt_d = nc.dram_tensor("xt", [SLOTS, D, B], _LDT, kind="ExternalInput").ap()
    pool_d = nc.dram_tensor(
        "pool", [SLOTS, N, D], dt.float32r, kind="ExternalInput"
    ).ap()
    poolt_d = nc.dram_tensor("poolt", [SLOTS, D, N], _LDT, kind="ExternalInput").ap()
    wy_d = nc.dram_tensor("wy", [SLOTS, D, D], _LDT, kind="ExternalInput").ap()
    bias_d = nc.dram_tensor(
        "bias", [SLOTS, 128, DC * A + DC], dt.float32, kind="ExternalInput"
    ).ap()
    ones_d = nc.dram_tensor("ones", [128, 1], dt.float32r, kind="ExternalInput").ap()
    out_d = nc.dram_tensor(
        "out", [B, SLOTS, A, D], dt.float32, kind="ExternalOutput"
    ).ap()

    with tile.TileContext(nc) as tc:
        with ExitStack() as ctx:
            p_const = ctx.enter_context(tc.tile_pool(name="p_const", bufs=1))
            p_wx = ctx.enter_context(tc.tile_pool(name="p_wx", bufs=2))
            p_xt = ctx.enter_context(tc.tile_pool(name="p_xt", bufs=2))
            p_pool = ctx.enter_context(tc.tile_pool(name="p_pool", bufs=2))
            p_poolt = ctx.enter_context(tc.tile_pool(name="p_poolt", bufs=2))
            p_wy = ctx.enter_context(tc.tile_pool(name="p_wy", bufs=2))
            p_bias = ctx.enter_context(tc.tile_pool(name="p_bias", bufs=2))
            p_xwxt = ctx.enter_context(tc.tile_pool(name="p_xwxt", bufs=2))
            p_ywyt = ctx.enter_context(tc.tile_pool(name="p_ywyt", bufs=1))
            p_expst = ctx.enter_context(tc.tile_pool(name="p_expst", bufs=1))
            p_soft = ctx.enter_context(tc.tile_pool(name="p_soft", bufs=2))
            p_out = ctx.enter_context(tc.tile_pool(name="p_out", bufs=2))
            ps_x = ctx.enter_context(tc.tile_pool(name="ps_x", bufs=2, space="PSUM"))
            ps_y = ctx.enter_context(tc.tile_pool(name="ps_y", bufs=2, space="PSUM"))
            ps_s = ctx.enter_context(tc.tile_pool(name="ps_s", bufs=2, space="PSUM"))
            ps_r = ctx.enter_context(tc.tile_pool(name="ps_r", bufs=1, space="PSUM"))
            ps_o = ctx.enter_context(tc.tile_pool(name="ps_o", bufs=1, space="PSUM"))

            ones_sb = p_const.tile([128, 1], dt.float32r)
            nc.sync.dma_start(ones_sb[:], ones_d[:])
            one_f32 = p_const.tile([1, 1], dt.float32)
            nc.vector.memset(one_f32[:], 1.0)

            for s in range(SLOTS):
                # ---- loads (wx first: stage A consumes it first) ----
                wx_sb = p_wx.tile([128, DC, A, D], _LDT, tag="wx")
                half = DC // 2
                nc.sync.dma_start(
                    wx_sb[:, 0:half],
                    wx_d[s, 0 : half * 128].rearrange("(c p) a e -> p c a e", p=128),
                )
                nc.scalar.dma_start(
                    wx_sb[:, half:DC],
                    wx_d[s, half * 128 : D].rearrange("(c p) a e -> p c a e", p=128),
                )
                xt_sb = p_xt.tile([128, DC, B], _LDT, tag="xt")
                nc.scalar.dma_start(
                    xt_sb[:], xt_d[s].rearrange("(c p) b -> p c b", p=128)
                )
                pool_sb = p_pool.tile([128, NK, D], dt.float32r, tag="pool")
                nc.sync.dma_start(
                    pool_sb[:, 0:7, :],
                    pool_d[s, 0 : 7 * 128].rearrange("(c p) d -> p c d", p=128),
                )
                nc.sync.dma_start(pool_sb[0:NLAST, 7, :], pool_d[s, 7 * 128 : N])
                poolt_sb = p_poolt.tile([128, DC, N], _LDT, tag="poolt")
                nc.scalar.dma_start(
                    poolt_sb[:], poolt_d[s].rearrange("(c p) n -> p c n", p=128)
                )
                wy_sb = p_wy.tile([128, DC, D], _LDT, tag="wy")
                nc.scalar.dma_start(
                    wy_sb[:], wy_d[s].rearrange("(c p) e -> p c e", p=128)
                )
                bias_sb = p_bias.tile([128, DC * A + DC], dt.float32, tag="bias")
                nc.sync.dma_start(bias_sb[:], bias_d[s])

                def wxb(ec, a):
                    return bias_sb[:, ec * A + a : ec * A + a + 1]

                def wyb(ec):
                    return bias_sb[:, DC * A + ec : DC * A + ec + 1]

                # ---- A: x_wxT[e, (a b)] = wx[a].T @ x.T + bias ----
                xwxt_sb = p_xwxt.tile([128, DC, A, B], dt.float32r, tag="xwxt")
                for ec in range(DC):
                    for a in range(A):
                        psx = ps_x.tile([128, B], dt.float32, tag="psx")
                        for c in range(DC):
                            nc.tensor.matmul(
                                psx[:],
                                wx_sb[:, c, a, ec * 128 : (ec + 1) * 128],
                                xt_sb[:, c, :],
                                start=(c == 0),
                                stop=(c == DC - 1),
                            )
                        nc.vector.tensor_scalar_add(
                            xwxt_sb[:, ec, a, :], psx[:], wxb(ec, a)
                        )

                # ---- B: y_wyT[e, n] = wy.T @ pool.T + bias ----
                ywyt_sb = p_ywyt.tile([128, DC, N], dt.float32r, tag="ywyt")
                for ec in range(DC):
                    for nh in range(2):
                        nsl = slice(nh * 500, (nh + 1) * 500)
                        psy = ps_y.tile([128, 500], dt.float32, tag="psy")
                        for c in range(DC):
                            nc.tensor.matmul(
                                psy[:],
                                wy_sb[:, c, ec * 128 : (ec + 1) * 128],
                                poolt_sb[:, c, nsl],
                                start=(c == 0),
                                stop=(c == DC - 1),
                            )
                        nc.vector.tensor_scalar_add(
                            ywyt_sb[:, ec, nsl], psy[:], wyb(ec)
                        )

                # ---- C: S^T[n, ba] + exp (unnormalized) + rowsum accum ----
                expst_sb = p_expst.tile([128, NK, BA], dt.float32r, tag="expst")
                for nk in range(NK):
                    pp = 128 if nk < 7 else NLAST
                    pss = ps_s.tile([128, BA], dt.float32, tag="pss")
                    for ec in range(DC):
                        nc.tensor.matmul(
                            pss[0:pp, :],
                            ywyt_sb[:, ec, nk * 128 : nk * 128 + pp],
                            xwxt_sb[:, ec, :, :],
                            start=(ec == 0),
                            stop=(ec == DC - 1),
                        )
                    nc.scalar.activation(
                        expst_sb[0:pp, nk, :],
                        pss[0:pp, :],
                        mybir.ActivationFunctionType.Exp,
                        scale=SCALE,
                    )

                # ---- D: rowsums over n via ones matmul ----
                psr = ps_r.tile([1, BA], dt.float32, tag="psr")
                for nk in range(NK):
                    pp = 128 if nk < 7 else NLAST
                    nc.tensor.matmul(
                        psr[:],
                        ones_sb[0:pp, :],
                        expst_sb[0:pp, nk, :],
                        start=(nk == 0),
                        stop=(nk == NK - 1),
                    )

                # ---- D: reciprocal -> partition-major ----
                rinv = p_soft.tile([1, BA], dt.float32, tag="rinv")
                nc.vector.reciprocal(rinv[:], psr[:])
                rsc = p_soft.tile([128, 3], dt.float32, tag="rsc")
                for bc in range(3):
                    pst = ps_r.tile([128, 1], dt.float32, tag="psr")
                    nc.tensor.transpose(
                        pst[:], rinv[0:1, bc * 128 : (bc + 1) * 128], one_f32[:]
                    )
                    nc.vector.tensor_copy(rsc[:, bc : bc + 1], pst[:])

                # ---- E: out[(a b), d] = (expS^T.T @ pool) * rinv ----
                outsb = p_out.tile([128, 3, D], dt.float32, tag="outsb")
                for bc in range(3):
                    pso = ps_o.tile([128, D], dt.float32, tag="pso")
                    for nk in range(NK):
                        pp = 128 if nk < 7 else NLAST
                        nc.tensor.matmul(
                            pso[:],
                            expst_sb[0:pp, nk, bc * 128 : (bc + 1) * 128],
                            pool_sb[0:pp, nk, :],
                            start=(nk == 0),
                            stop=(nk == NK - 1),
                        )
                    nc.vector.tensor_scalar_mul(
                        outsb[:, bc, :], pso[:], rsc[:, bc : bc + 1]
                    )
                    nc.scalar.dma_start(
                        out_d[:, s, 2 * bc : 2 * bc + 2, :].rearrange(
                            "b a d -> a b d"
                        ),
                        outsb[:, bc, :],
                    )

    nc.compile()
    return nc


def _prep_inputs(
    top_region_features, normality_pool_image_features, wx, wx_bias, wy, wy_bias
):
    x = _np(top_region_features).astype(np.float32)
    pool = _np(normality_pool_image_features).astype(np.float32)
    wx = _np(wx).astype(np.float32)
    wxb = _np(wx_bias).astype(np.float32)
    wy = _np(wy).astype(np.float32)
    wyb = _np(wy_bias).astype(np.float32)

    rmap = list(range(R)) + [0, 1, 2]
    ones = np.ones((128, 1), np.float32)
    in_maps = []
    for i in range(N_CORES):
        regs = rmap[SLOTS * i : SLOTS * (i + 1)]
        # bias pack: [slot, partition(=e%128), DC*A (wx) + DC (wy)]
        wxb_s = (
            wxb[:, regs, 0, :]  # (A, S, D)
            .transpose(1, 2, 0)  # (S, D, A)
            .reshape(SLOTS, DC, 128, A)
            .transpose(0, 2, 1, 3)  # (S, 128, DC, A)
            .reshape(SLOTS, 128, DC * A)
        )
        wyb_s = (
            wyb[regs, 0, :].reshape(SLOTS, DC, 128).transpose(0, 2, 1)
        )  # (S, 128, DC)
        bias = np.concatenate([wxb_s, wyb_s], axis=2)  # (S, 128, DC*A+DC)
        in_maps.append(
            {
                "wx": _np(wx[:, regs].transpose(1, 2, 0, 3)).astype(_LNP),
                "xt": _np(x[:, regs].transpose(1, 2, 0)).astype(_LNP),
                "pool": _np(pool[regs]),
                "poolt": _np(pool[regs].transpose(0, 2, 1)).astype(_LNP),
                "wy": _np(wy[regs]).astype(_LNP),
                "bias": _np(bias),
                "ones": ones,
            }
        )
    return in_maps


def _gather(results):
    full = np.empty((B, R, A, D), np.float32)
    for j in range(R):
        full[:, j] = results[j // SLOTS]["out"][:, j % SLOTS]
    return full


def _get_nc():
    if "nc" not in _CACHE:
        _CACHE["nc"] = _build()
    return _CACHE["nc"]


def run(in_maps, **kw):
    nc = _get_nc()
    return run_bass_kernel_spmd(nc, in_maps, list(range(N_CORES)), **kw)


def kernel(**inputs):
    in_maps = _prep_inputs(**inputs)
    res = run(in_maps)
    return _gather(res.results)


if __name__ == "__main__":
    rng = np.random.default_rng(0)
    s = 1.0 / np.sqrt(D)
    inputs = {
        "top_region_features": rng.standard_normal((B, R, D), np.float32),
        "normality_pool_image_features": rng.standard_normal((R, N, D), np.float32),
        "wx": (rng.standard_normal((A, R, D, D)) * s).astype(np.float32),
        "wx_bias": (rng.standard_normal((A, R, 1, D)) * s).astype(np.float32),
        "wy": (rng.standard_normal((R, D, D)) * s).astype(np.float32),
        "wy_bias": (rng.standard_normal((R, 1, D)) * s).astype(np.float32),
    }
    out = kernel(**inputs)
    print("out", out.shape, out.dtype, float(np.abs(out).mean()))
